# revision 1
# baseline (speedup 1.0000x reference)
"""DerivativeNet (direction='x') on 8 Trainium2 NeuronCores.

Contract: kernel(u, mask) takes FULL inputs
  u    [16, 2, 1024, 1024] f32
  mask [16, 1, 1024, 1024] f32
returns FULL output [16, 2, 1024, 1024] f32.

Sharding: pure data-parallel over batch — 2 samples per core, 8 cores.

Per-row math along W (h = 0.01, zero-padded):
  d[k]   = up[k+1] - up[k]           (up = [0, u, 0])
  out[w] = p'[w]*d[w+1] + q'[w]*d[w]
  p' = eroded/(2h) + (cs==1)/h
  q' = eroded/(2h) + ((cs==total)&m)/h
  eroded = (cs[w+1]-cs[w-2] == 3),  cs = cumsum(m) along w

The mask pipeline runs in fp16 (values are small integers, exact in fp16
up to 2048 >= W). The u data path stays fp32.
"""

import sys

if "/opt/trn_rl_repo" not in sys.path:
    sys.path.insert(0, "/opt/trn_rl_repo")

import numpy as np

_B, _C, _H, _W = 16, 2, 1024, 1024
_NCORES = 8
_BS = _B // _NCORES              # batch per core
_INV_H = 100.0
_INV_2H = 50.0

# engine/tuning configuration.  HW-measured (8 axon TRN2 cores):
# - GpSimd elementwise work serializes against DVE (shared SBUF port
#   lock), so ALL elementwise work lives on DVE (+ACT for 1-src ops).
# - PE identity-matmul adds are exact but fp32 weights self-load per
#   matmul (no FWL) making them slower than the DVE add they replace.
# - DMA ring-splitting (out-stores on the ACT HWDGE ring) speeds up
#   DMA-only runs, but ACT-seq DMA issue stalls ACT compute in the full
#   kernel, so everything stays on the sync ring (DMA ~200us/core hides
#   under the ~215us DVE-bound compute).
CFG = dict(
    S=2,                 # h-segments per SBUF tile
    bufs=3,              # tile pool buffers
    # u-side pass engines: "dve" or "gp"; fadd also "pe"
    dsub="dve",
    fadd="dve",
    # how many of the 16 u-tiles route dsub/fadd to gpsimd (rest dve)
    dsub_gp_frac=0.0,
    fadd_gp_frac=0.0,
    box="dve",
    pq="dve",            # pco-add/qco-mult/qco-add engine
    mul1="dve",          # t1 = p'*d1 engine
    mul2="dve",          # t2 = q'*d2 engine
    er_act=False,        # eroded via ACT relu instead of DVE ts
    pco_act=False,       # (cs==1) via ACT abs+relu instead of DVE ts
    iters=1,             # benchmark mode: repeat whole body in a HW loop
    dma_only=False,      # benchmark mode: only DMAs, no compute
    merge_c=False,       # (broken: 4D DMA unbalanceable; keep False)
    out_eng="sync",      # ACT-ring DMA issue stalls ACT compute; keep sync
    ubufs=2,             # buffers for the big u-side tiles
    uload_act_frac=0.0,  # fraction of u-load DMAs issued on the ACT ring
    er_stt=False,        # fuse box-diff + erosion threshold into one STT
    pe_dtype="f32",      # identity weight dtype for the PE add ("bf16"/"f32")
    t1_inplace=False,    # write t1 into the up tile (saves SBUF, hurts overlap)
    pads_act=False,      # zero the pad columns via ACT memzero instead of DVE
)

_CACHE = {}


def _build_nc(cfg=None):
    import concourse.tile as tile
    from concourse import bacc, mybir

    cfg = dict(CFG, **(cfg or {}))
    F32 = mybir.dt.float32
    F16 = mybir.dt.float16
    Alu = mybir.AluOpType

    nc = bacc.Bacc("TRN2", target_bir_lowering=False, debug=False,
                   enable_asserts=False, num_devices=_NCORES)
    u_ap = nc.dram_tensor("u", [_BS, _C, _H, _W], F32,
                          kind="ExternalInput").ap()
    m_ap = nc.dram_tensor("mask", [_BS, _H, _W], F32,
                          kind="ExternalInput").ap()
    o_ap = nc.dram_tensor("out", [_BS, _C, _H, _W], F32,
                          kind="ExternalOutput").ap()

    P, S, W = 128, cfg["S"], _W
    R = P * S
    HT = _H // R
    Wp = W + 4
    Wu = W + 2
    NU = _BS * HT * _C           # total u tiles

    def eng(name):
        return {"dve": nc.vector, "gp": nc.gpsimd}[cfg[name]]

    use_pe = cfg["fadd"] == "pe"

    with tile.TileContext(nc) as tc:
        with _stack() as ctx:
            pool = ctx.enter_context(tc.tile_pool(name="dn", bufs=cfg["bufs"]))
            upool = ctx.enter_context(tc.tile_pool(name="du",
                                                   bufs=cfg["ubufs"]))
            cpool = ctx.enter_context(tc.tile_pool(name="cn", bufs=1))

            def const_col(val):
                t = cpool.tile([P, 1], F32, tag=f"c{val}")
                nc.vector.memset(t[:], val)
                return t

            if cfg["er_act"]:
                bias_er = const_col(-2.0 * _INV_2H)
            if cfg["pco_act"]:
                bias_m1 = const_col(-1.0)
                bias_ph = const_col(_INV_H)
            if use_pe:
                ppool = ctx.enter_context(
                    tc.tile_pool(name="ps", bufs=2, space="PSUM"))
                id_dt = (mybir.dt.bfloat16 if cfg["pe_dtype"] == "bf16"
                         else F32)
                id_ap = nc.dram_tensor("ident", [P, P], id_dt,
                                       kind="ExternalInput").ap()
                ident = cpool.tile([P, P], id_dt, tag="ident")
                nc.sync.dma_start(ident[:], id_ap)

            if cfg["iters"] > 1:
                loop_cm = tc.For_i(0, cfg["iters"], 1)
                ctx.enter_context(loop_cm)
            e_odma = nc.scalar if cfg["out_eng"] == "scalar" else nc.sync
            uidx = 0
            if cfg["dma_only"]:
                for b in range(_BS):
                    for ht in range(HT):
                        r0 = ht * R
                        m32 = pool.tile([P, S, W], F32, tag="m32")
                        msrc = m_ap[b, r0:r0 + R, :].rearrange(
                            "(s p) w -> p s w", p=P)
                        nc.sync.dma_start(m32[:], msrc)
                        if cfg["merge_c"]:
                            up2 = upool.tile([P, _C, S, W], F32, tag="up2")
                            usrc = u_ap[b, :, r0:r0 + R, :].rearrange(
                                "c (s p) w -> p c s w", p=P)
                            nc.sync.dma_start(up2[:], usrc)
                            odst = o_ap[b, :, r0:r0 + R, :].rearrange(
                                "c (s p) w -> p c s w", p=P)
                            e_odma.dma_start(odst, up2[:])
                        else:
                            for c in range(_C):
                                up = upool.tile([P, S, W], F32, tag="up")
                                usrc = u_ap[b, c, r0:r0 + R, :].rearrange(
                                    "(s p) w -> p s w", p=P)
                                e_ul = (nc.scalar if uidx <
                                        cfg["uload_act_frac"] * NU
                                        else nc.sync)
                                e_ul.dma_start(up[:], usrc)
                                odst = o_ap[b, c, r0:r0 + R, :].rearrange(
                                    "(s p) w -> p s w", p=P)
                                e_odma.dma_start(odst, up[:])
                                uidx += 1
            for b in ([] if cfg["dma_only"] else range(_BS)):
                for ht in range(HT):
                    r0 = ht * R
                    m32 = pool.tile([P, S, W], F32, tag="m32")
                    msrc = m_ap[b, r0:r0 + R, :].rearrange(
                        "(s p) w -> p s w", p=P)
                    nc.sync.dma_start(m32[:], msrc)
                    mf = pool.tile([P, S, W], F16, tag="mf")
                    nc.scalar.copy(mf[:], m32[:])

                    csp = pool.tile([P, S, Wp], F16, tag="csp")
                    if cfg["pads_act"]:
                        nc.scalar.memzero(csp[:, :, 0:2])
                    else:
                        nc.vector.memset(csp[:, :, 0:2], 0.0)
                    for s in range(S):
                        nc.vector.tensor_tensor_scan(
                            csp[:, s, 2:2 + W], mf[:, s, :], mf[:, s, :],
                            0.0, Alu.add, Alu.bypass)
                    nc.scalar.copy(csp[:, :, 2 + W:3 + W],
                                   csp[:, :, 1 + W:2 + W])

                    cs = csp[:, :, 2:2 + W]
                    er = pool.tile([P, S, W], F16, tag="er")
                    if cfg["er_stt"]:
                        # er = (csp[w+3] - 2.5 >= csp[w]) = eroded (0/1)
                        nc.vector.scalar_tensor_tensor(
                            er[:], csp[:, :, 3:3 + W], -2.5,
                            csp[:, :, 0:W], Alu.add, Alu.is_ge)
                    else:
                        box = pool.tile([P, S, W], F16, tag="box")
                        eng("box").tensor_sub(box[:], csp[:, :, 3:3 + W],
                                              csp[:, :, 0:W])
                        if cfg["er_act"]:
                            nc.scalar.activation(
                                er[:], box[:],
                                mybir.ActivationFunctionType.Relu,
                                bias=bias_er[:], scale=_INV_2H)
                        else:
                            nc.vector.tensor_scalar(er[:], box[:], 2.5,
                                                    _INV_2H,
                                                    Alu.is_ge, Alu.mult)
                    pco = pool.tile([P, S, W], F16, tag="pco")
                    if cfg["pco_act"]:
                        nc.scalar.activation(
                            pco[:], cs, mybir.ActivationFunctionType.Abs,
                            bias=bias_m1[:])
                        nc.scalar.activation(
                            pco[:], pco[:],
                            mybir.ActivationFunctionType.Relu,
                            bias=bias_ph[:], scale=-2.0 * _INV_H)
                    else:
                        nc.vector.tensor_scalar(pco[:], cs, 1.0, _INV_H,
                                                Alu.is_equal, Alu.mult)
                    if cfg["er_stt"]:
                        nc.vector.scalar_tensor_tensor(
                            pco[:], er[:], _INV_2H, pco[:],
                            Alu.mult, Alu.add)
                    else:
                        eng("pq").tensor_add(pco[:], pco[:], er[:])
                    tot32 = pool.tile([P, S, 1], F32, tag="tot32")
                    nc.scalar.copy(tot32[:], csp[:, :, 1 + W:2 + W])
                    qco = pool.tile([P, S, W], F16, tag="qco")
                    for s in range(S):
                        nc.vector.tensor_scalar(
                            qco[:, s, :], csp[:, s, 2:2 + W],
                            tot32[:, s, :], _INV_H,
                            Alu.is_equal, Alu.mult)
                    eng("pq").tensor_mul(qco[:], qco[:], mf[:])
                    if cfg["er_stt"]:
                        nc.vector.scalar_tensor_tensor(
                            qco[:], er[:], _INV_2H, qco[:],
                            Alu.mult, Alu.add)
                    else:
                        eng("pq").tensor_add(qco[:], qco[:], er[:])

                    if cfg["merge_c"]:
                        e_dsub = (nc.gpsimd if (cfg["dsub"] == "gp" and
                                  uidx < cfg["dsub_gp_frac"] * NU)
                                  else nc.vector)
                        e_fadd = (nc.gpsimd if (cfg["fadd"] == "gp" and
                                  uidx < cfg["fadd_gp_frac"] * NU)
                                  else nc.vector)
                        up2 = upool.tile([P, _C, S, Wu], F32, tag="up2")
                        nc.vector.memset(up2[:, :, :, 0:1], 0.0)
                        nc.vector.memset(up2[:, :, :, W + 1:W + 2], 0.0)
                        usrc = u_ap[b, :, r0:r0 + R, :].rearrange(
                            "c (s p) w -> p c s w", p=P)
                        nc.sync.dma_start(up2[:, :, :, 1:1 + W], usrc)
                        d2 = upool.tile([P, _C, S, Wu], F32, tag="d2")
                        e_dsub.tensor_sub(d2[:, :, :, 0:W + 1],
                                          up2[:, :, :, 1:W + 2],
                                          up2[:, :, :, 0:W + 1])
                        t12 = upool.tile([P, _C, S, W], F32, tag="t12")
                        for c in range(_C):
                            eng("mul1").tensor_mul(t12[:, c], pco[:],
                                                   d2[:, c, :, 1:1 + W])
                            eng("mul2").tensor_mul(d2[:, c, :, 0:W], qco[:],
                                                   d2[:, c, :, 0:W])
                        e_fadd.tensor_add(t12[:], t12[:],
                                          d2[:, :, :, 0:W])
                        odst = o_ap[b, :, r0:r0 + R, :].rearrange(
                            "c (s p) w -> p c s w", p=P)
                        e_odma.dma_start(odst, t12[:])
                        uidx += 2
                        continue
                    for c in range(_C):
                        e_dsub = (nc.gpsimd if (cfg["dsub"] == "gp" and
                                  uidx < cfg["dsub_gp_frac"] * NU)
                                  else nc.vector)
                        up = upool.tile([P, S, Wu], F32, tag="up")
                        if cfg["pads_act"]:
                            nc.scalar.memzero(up[:, :, 0:1])
                            nc.scalar.memzero(up[:, :, W + 1:W + 2])
                        else:
                            nc.vector.memset(up[:, :, 0:1], 0.0)
                            nc.vector.memset(up[:, :, W + 1:W + 2], 0.0)
                        usrc = u_ap[b, c, r0:r0 + R, :].rearrange(
                            "(s p) w -> p s w", p=P)
                        e_ul = (nc.scalar if (uidx % 4) <
                                cfg["uload_act_frac"] * 4 else nc.sync)
                        e_ul.dma_start(up[:, :, 1:1 + W], usrc)
                        d = upool.tile([P, S, Wu], F32, tag="d")
                        e_dsub.tensor_sub(d[:, :, 0:W + 1],
                                          up[:, :, 1:W + 2],
                                          up[:, :, 0:W + 1])
                        if cfg["t1_inplace"]:
                            t1 = up[:, :, 0:W]
                        else:
                            t1t = upool.tile([P, S, W], F32, tag="t1")
                            t1 = t1t[:]
                        eng("mul1").tensor_mul(t1, pco[:],
                                               d[:, :, 1:1 + W])
                        eng("mul2").tensor_mul(d[:, :, 0:W], qco[:],
                                               d[:, :, 0:W])
                        odst = o_ap[b, c, r0:r0 + R, :].rearrange(
                            "(s p) w -> p s w", p=P)
                        if use_pe:
                            pt = ppool.tile([P, S, W], F32, tag="pt")
                            for s in range(S):
                                for j in range(0, W, 512):
                                    nc.tensor.matmul(
                                        pt[:, s, j:j + 512], ident[:],
                                        t1[:, s, j:j + 512],
                                        start=True, stop=False)
                                    nc.tensor.matmul(
                                        pt[:, s, j:j + 512], ident[:],
                                        d[:, s, j:j + 512],
                                        start=False, stop=True)
                            ot = upool.tile([P, S, W], F32, tag="ot")
                            nc.scalar.copy(ot[:], pt[:])
                            e_odma.dma_start(odst, ot[:])
                        else:
                            e_fadd = (nc.gpsimd if (cfg["fadd"] == "gp" and
                                      uidx < cfg["fadd_gp_frac"] * NU)
                                      else nc.vector)
                            e_fadd.tensor_add(t1, t1, d[:, :, 0:W])
                            e_odma.dma_start(odst, t1)
                        uidx += 1
    nc.compile()
    return nc


def _stack():
    from contextlib import ExitStack
    return ExitStack()


def _get_runner():
    """Build, compile and jit once; return a callable
    (u_full, mask_full) -> out_full that just executes."""
    if "runner" in _CACHE:
        return _CACHE["runner"]

    import jax
    from jax.sharding import Mesh, PartitionSpec
    from jax.experimental.shard_map import shard_map
    from concourse import bass2jax, mybir

    nc = _build_nc()
    bass2jax.install_neuronx_cc_hook()

    partition_name = (nc.partition_id_tensor.name
                      if nc.partition_id_tensor else None)
    in_names = []
    out_names = []
    out_avals = []
    zero_shapes = []
    for alloc in nc.m.functions[0].allocations:
        if not isinstance(alloc, mybir.MemoryLocationSet):
            continue
        name = alloc.memorylocations[0].name
        if alloc.kind == "ExternalInput":
            if name != partition_name:
                in_names.append(name)
        elif alloc.kind == "ExternalOutput":
            out_names.append(name)
            shape = tuple(alloc.tensor_shape)
            dtype = mybir.dt.np(alloc.dtype)
            out_avals.append(jax.core.ShapedArray(shape, dtype))
            zero_shapes.append((shape, dtype))
    n_params = len(in_names)
    all_names = in_names + out_names
    if partition_name is not None:
        all_names = all_names + [partition_name]

    def _body(*args):
        operands = list(args)
        if partition_name is not None:
            operands.append(bass2jax.partition_id_tensor())
        outs = bass2jax._bass_exec_p.bind(
            *operands,
            out_avals=tuple(out_avals),
            in_names=tuple(all_names),
            out_names=tuple(out_names),
            lowering_input_output_aliases=(),
            sim_require_finite=True,
            sim_require_nnan=True,
            nc=nc,
        )
        return tuple(outs)

    devices = jax.devices()[:_NCORES]
    mesh = Mesh(np.asarray(devices), ("core",))
    n_outs = len(out_names)
    sharded = jax.jit(
        shard_map(_body, mesh=mesh,
                  in_specs=(PartitionSpec("core"),) * (n_params + n_outs),
                  out_specs=(PartitionSpec("core"),) * n_outs,
                  check_rep=False),
        donate_argnums=tuple(range(n_params, n_params + n_outs)),
        keep_unused=True,
    )

    name_to_idx = {n: i for i, n in enumerate(in_names)}

    def run(u_full, mask_full):
        u_full = np.ascontiguousarray(u_full, dtype=np.float32)
        mask_full = np.ascontiguousarray(
            mask_full, dtype=np.float32).reshape(_B, _H, _W)
        # per-core shard along axis 0 = declared per-core shape, so the
        # [16, ...] batch-major arrays are already the global view
        args = [None] * n_params
        args[name_to_idx["u"]] = u_full
        args[name_to_idx["mask"]] = mask_full
        if "ident" in name_to_idx:
            import ml_dtypes
            idt = (ml_dtypes.bfloat16 if CFG.get("pe_dtype") == "bf16"
                   else np.float32)
            args[name_to_idx["ident"]] = np.tile(
                np.eye(128, dtype=idt), (_NCORES, 1))
        zeros = [np.zeros((_NCORES * s[0], *s[1:]), d)
                 for (s, d) in zero_shapes]
        out_arrs = sharded(*args, *zeros)
        out = np.asarray(out_arrs[out_names.index("out")])
        return out.reshape(_B, _C, _H, _W)

    _CACHE["runner"] = run
    return run


def kernel(u, mask):
    run = _get_runner()
    return run(u, mask)


if __name__ == "__main__":
    rng = np.random.default_rng(0)
    u = rng.standard_normal((_B, _C, _H, _W), dtype=np.float32)
    mask = (rng.random((_B, 1, _H, _W)) < 0.5).astype(np.float32)
    out = kernel(u=u, mask=mask)
    print("out", out.shape, out.dtype, float(np.abs(out).max()))



# revision 4
# speedup vs baseline: 1.4967x; 1.4967x over previous
"""DerivativeNet (direction='x') on 8 TRN2 cores — v3 (16-bit datapath).

Contract: kernel(u, mask) takes FULL inputs
  u    [16, 2, 1024, 1024] f32
  mask [16, 1, 1024, 1024] f32
returns FULL output [16, 2, 1024, 1024] f32.
Sharding: pure data-parallel over batch — 2 samples per core, 8 cores.

Math per row along W (h=0.01, zero-padded):
  d[k]   = u[k] - u[k-1]          k = 0..W   (u[-1] = u[W] = 0)
  out[w] = pco[w]*d[w+1] + qco[w]*d[w]
  pco = er/2h + (cs==1)/h
  qco = er/2h + ((cs==tot)&m)/h
  er  = (cs[w+1]-cs[w-2] == 3),  cs = cumsum(m) along w

Design (HW-measured ~200us/iter vs 275-330us for the fp32 v1):
  - whole datapath in f16 (u ~N(0,1); out scale ~500; f16 rel err ~5e-4
    << 2e-2 gate).  16-bit DVE tensor_tensor runs 2x / tensor_scalar 4x
    when slices are 4-byte aligned.
  - mask is host-converted to f16 (binary 0/1 — lossless) so it loads
    directly as the compute dtype and mask DMA bytes halve.
  - u loads stay f32; the f32->f16 cast rides the dsub (f32 in, f16
    out), costing nothing extra.
  - DMA split over both HWDGE rings: u+mask loads on the ACT ring,
    out stores on the sync ring.
  - alignment tricks (odd shifts break DVE 2x packing): ScalarE makes
    1-shifted copies csB = csp[k+1]-2.5 (eroded test becomes an aligned
    TT sub + ScalarE relu -> er50 pre-scaled) and pcoS = pco[k-1]
    (t1 multiply runs 2x aligned); pcoS reuses csB's buffer.
  - final add out[w] = t1[w+1] + t2[w] has an unavoidable odd shift ->
    done on the PE as two f16 identity matmuls accumulating in PSUM
    (fp32), which upcasts to f32 for free; ScalarE copies PSUM->SBUF.
  - GPSIMD/SWDGE deliberately unused: SWDGE cast-DMAs measured ~92GB/s
    (vs ~210+ HWDGE) and Q7 elementwise serializes against DVE 2-port
    ops (shared SBUF port lock).
"""

import sys

if "/opt/trn_rl_repo" not in sys.path:
    sys.path.insert(0, "/opt/trn_rl_repo")

import numpy as np

_B, _C, _H, _W = 16, 2, 1024, 1024
_NCORES = 8
_BS = _B // _NCORES
_INV_H = 100.0
_INV_2H = 50.0

CFG = CFG2 = dict(
    S=4,                # h-segments per tile (tile rows = 128*S)
    bufs=2,             # mask-side pool buffers
    ubufs=2,            # u-side pool buffers
    obufs=2,            # out-staging pool buffers
    u_load="scalar",    # HWDGE ring for u loads ("sync"/"scalar")
    m_load="scalar",    # "gp" = SWDGE cast f32->f16 | "sync"/"scalar"
    m_dtype="f16",      # "f32" | "f16": declared HBM dtype of mask (host
                        # converts; binary mask is exact in f16)
    out_eng="sync",     # HWDGE ring for out stores
    u_conv="fold",      # "fold": dsub reads f32, writes f16 (free cast)
                        # | "gp"|"scalar"|"dve": explicit convert engine
    m_conv="gp",        # (only when m_load != "gp" and m_dtype == "f32")
    er_mode="relu",     # "relu": TT box-sub + ScalarE relu -> er50
                        # | "stt": scalar_tensor_tensor compare (1x on DVE)
    scan_eng="dve",     # cumsum engine: "dve" | "gp" (frees ~20us of DVE;
                        # Q7 dependent-chain scan is slower but idle)
    pads_once=False,    # zero pad columns once per pool buffer pre-loop
                        # (they are never overwritten) instead of per tile
    er_csb=True,        # ScalarE shifted copy of csp -> eroded test 2x
    pcos=True,          # ScalarE shifted copy of pco -> t1 mul 2x
    t1_alias=True,      # write t1 into the up tile (f16 u path only)
    fadd="pe",          # "pe" identity-matmul add | "dve32" f32-out add
    iters=1,            # hardware-loop repeat (benchmark mode)
    mode="full",        # "full" | "dma" (loads+stores only)
)


def _build_nc(cfg=None):
    import concourse.tile as tile
    from concourse import bacc, mybir

    cfg = dict(CFG2, **(cfg or {}))
    F32 = mybir.dt.float32
    F16 = mybir.dt.float16

    nc = bacc.Bacc("TRN2", target_bir_lowering=False, debug=False,
                   enable_asserts=False, num_devices=_NCORES)
    u_ap = nc.dram_tensor("u", [_BS, _C, _H, _W], F32,
                          kind="ExternalInput").ap()
    m_dt = F16 if cfg["m_dtype"] == "f16" else F32
    m_ap = nc.dram_tensor("mask", [_BS, _H, _W], m_dt,
                          kind="ExternalInput").ap()
    o_ap = nc.dram_tensor("out", [_BS, _C, _H, _W], F32,
                          kind="ExternalOutput").ap()

    P, S, W = 128, cfg["S"], _W
    R = P * S
    HT = _H // R
    use_pe = cfg["fadd"] == "pe" and cfg["mode"] == "full"

    with tile.TileContext(nc) as tc:
        with _stack() as ctx:
            pool = ctx.enter_context(tc.tile_pool(name="mp",
                                                  bufs=cfg["bufs"]))
            upool = ctx.enter_context(tc.tile_pool(name="up",
                                                   bufs=cfg["ubufs"]))
            opool = ctx.enter_context(tc.tile_pool(name="op",
                                                   bufs=cfg["obufs"]))
            cpool = ctx.enter_context(tc.tile_pool(name="cp", bufs=1))
            consts = {}
            if cfg["mode"] == "full" and cfg["er_mode"] == "relu":
                for val, nm in ((-2.5, "m25"), (0.0, "z")):
                    t = cpool.tile([P, 1], mybir.dt.float32, tag=f"c{nm}")
                    nc.vector.memset(t[:], val)
                    consts[nm] = t
            ppool = None
            ident = None
            if use_pe:
                ppool = ctx.enter_context(
                    tc.tile_pool(name="pp", bufs=2, space="PSUM"))
                id_ap = nc.dram_tensor("ident", [P, P], F16,
                                       kind="ExternalInput").ap()
                ident = cpool.tile([P, P], F16, tag="ident")
                nc.sync.dma_start(ident[:], id_ap)

            if (cfg["mode"] == "full" and cfg["pads_once"]
                    and cfg["u_conv"] == "fold"):
                # Pad columns are never overwritten by loads/compute, so
                # zero them once per rotating pool buffer, outside the
                # hardware loop.
                for _ in range(cfg["ubufs"]):
                    t = upool.tile([P, S, W + 2], F32, tag="u32p")
                    nc.scalar.memzero(t[:, :, 0:1])
                    nc.scalar.memzero(t[:, :, 1 + W:2 + W])
                for _ in range(cfg["bufs"]):
                    t = pool.tile([P, S, W + 4], F16, tag="csp")
                    nc.scalar.memzero(t[:, :, 0:2])

            if cfg["iters"] > 1:
                loop_cm = tc.For_i(0, cfg["iters"], 1)
                ctx.enter_context(loop_cm)

            if cfg["mode"] == "dma":
                _body_dma(nc, cfg, pool, upool, u_ap, m_ap, o_ap,
                          P, S, W, R, HT)
            else:
                _body(nc, cfg, pool, upool, opool, ppool, ident, consts,
                      u_ap, m_ap, o_ap, P, S, W, R, HT)
    nc.compile()
    return nc


def _body_dma(nc, cfg, pool, upool, u_ap, m_ap, o_ap, P, S, W, R, HT):
    from concourse import mybir

    F32 = mybir.dt.float32
    F16 = mybir.dt.float16
    e_odma = nc.scalar if cfg["out_eng"] == "scalar" else nc.sync
    for b in range(_BS):
        for ht in range(HT):
            r0 = ht * R
            msrc = m_ap[b, r0:r0 + R, :].rearrange("(s p) w -> p s w", p=P)
            if cfg["m_dtype"] == "f16":
                mf = pool.tile([P, S, W], F16, tag="mf")
                eng = nc.scalar if cfg["m_load"] in ("scalar", "gp") \
                    else nc.sync
                eng.dma_start(mf[:], msrc)
            elif cfg["m_load"] == "gp":
                mf = pool.tile([P, S, W], F16, tag="mf")
                nc.gpsimd.dma_start(mf[:], msrc)
            else:
                m32 = pool.tile([P, S, W], F32, tag="m32")
                eng = nc.sync if cfg["m_load"] == "sync" else nc.scalar
                eng.dma_start(m32[:], msrc)
            for c in range(_C):
                usrc = u_ap[b, c, r0:r0 + R, :].rearrange(
                    "(s p) w -> p s w", p=P)
                odst = o_ap[b, c, r0:r0 + R, :].rearrange(
                    "(s p) w -> p s w", p=P)
                if cfg["u_load"] == "gp":
                    uf = upool.tile([P, S, W], F16, tag="uf")
                    nc.gpsimd.dma_start(uf[:], usrc)
                    nc.gpsimd.dma_start(odst, uf[:])
                else:
                    u32 = upool.tile([P, S, W], F32, tag="u32")
                    eng = nc.sync if cfg["u_load"] == "sync" else nc.scalar
                    eng.dma_start(u32[:], usrc)
                    e_odma.dma_start(odst, u32[:])


def _body(nc, cfg, pool, upool, opool, ppool, ident, consts,
          u_ap, m_ap, o_ap, P, S, W, R, HT):
    from concourse import mybir

    F32 = mybir.dt.float32
    F16 = mybir.dt.float16
    Alu = mybir.AluOpType
    use_pe = cfg["fadd"] == "pe"
    e_odma = nc.scalar if cfg["out_eng"] == "scalar" else nc.sync
    e_uload = nc.sync if cfg["u_load"] == "sync" else nc.scalar
    Wp = W + 4

    for b in range(_BS):
        for ht in range(HT):
            r0 = ht * R

            # ---- mask pipeline (shared across channels) ----
            msrc = m_ap[b, r0:r0 + R, :].rearrange("(s p) w -> p s w", p=P)
            if cfg["m_dtype"] == "f16":
                mf = pool.tile([P, S, W], F16, tag="mf")
                eng = nc.scalar if cfg["m_load"] in ("scalar", "gp") \
                    else nc.sync
                eng.dma_start(mf[:], msrc)
            elif cfg["m_load"] == "gp":
                mf = pool.tile([P, S, W], F16, tag="mf")
                nc.gpsimd.dma_start(mf[:], msrc)
            else:
                m32 = pool.tile([P, S, W], F32, tag="m32")
                eng = nc.sync if cfg["m_load"] == "sync" else nc.scalar
                eng.dma_start(m32[:], msrc)
                mf = pool.tile([P, S, W], F16, tag="mf")
                if cfg["m_conv"] == "gp":
                    nc.gpsimd.tensor_copy(mf[:], m32[:])
                else:
                    nc.scalar.copy(mf[:], m32[:])

            # csp: [pad pad cs[0..W-1] edge spare], edge = cs[W-1]
            csp = pool.tile([P, S, Wp], F16, tag="csp")
            if not (cfg["pads_once"] and cfg["u_conv"] == "fold"):
                nc.scalar.memzero(csp[:, :, 0:2])
            e_scan = nc.gpsimd if cfg["scan_eng"] == "gp" else nc.vector
            for s in range(S):
                e_scan.tensor_tensor_scan(
                    csp[:, s, 2:2 + W], mf[:, s, :], mf[:, s, :],
                    0.0, Alu.add, Alu.bypass)
            nc.scalar.copy(csp[:, :, 2 + W:3 + W], csp[:, :, 1 + W:2 + W])
            tot32 = pool.tile([P, S, 1], F32, tag="tot32")
            nc.scalar.copy(tot32[:], csp[:, :, 1 + W:2 + W])

            # eroded, pre-scaled: er50[w] = (cs[w+1]-cs[w-2] == 3) / 2h
            er = pool.tile([P, S, W], F16, tag="er")
            if cfg["er_mode"] == "relu":
                # csB[k] = csp[k+1] - 2.5 (ScalarE biased shift), then
                # box25 = csB[w+2] - csp[w] in {-2.5..0.5} (TT, 2x),
                # er50 = relu(100*box25) in {0, 50} (ScalarE, off-DVE).
                csB = pool.tile([P, S, W + 2], F16, tag="csB")
                nc.scalar.activation(csB[:], csp[:, :, 1:3 + W],
                                     mybir.ActivationFunctionType.Identity,
                                     bias=consts['m25'][:])
                nc.vector.tensor_sub(er[:], csB[:, :, 2:2 + W],
                                     csp[:, :, 0:W])
                # box25 in {-2.5,...,0.5}: relu(100*box25) -> {0, 50}
                nc.scalar.activation(er[:], er[:],
                                     mybir.ActivationFunctionType.Relu,
                                     bias=consts['z'][:],
                                     scale=2.0 * _INV_2H)
            elif cfg["er_csb"]:
                csB = pool.tile([P, S, W + 2], F16, tag="csB")
                nc.scalar.copy(csB[:], csp[:, :, 1:3 + W])
                nc.vector.scalar_tensor_tensor(
                    er[:], csB[:, :, 2:2 + W], -2.5, csp[:, :, 0:W],
                    Alu.add, Alu.is_ge)
                nc.vector.tensor_scalar_mul(er[:], er[:], _INV_2H)
            else:
                csB = None
                nc.vector.scalar_tensor_tensor(
                    er[:], csp[:, :, 3:3 + W], -2.5, csp[:, :, 0:W],
                    Alu.add, Alu.is_ge)
                nc.vector.tensor_scalar_mul(er[:], er[:], _INV_2H)

            # pco = er50 + (cs==1)/h
            pco = pool.tile([P, S, W], F16, tag="pco")
            nc.vector.tensor_scalar(pco[:], csp[:, :, 2:2 + W], 1.0,
                                    _INV_H, Alu.is_equal, Alu.mult)
            nc.vector.tensor_add(pco[:], pco[:], er[:])

            # qco = er50 + ((cs==tot)&m)/h
            qco = pool.tile([P, S, W], F16, tag="qco")
            for s in range(S):
                nc.vector.tensor_scalar(
                    qco[:, s, :], csp[:, s, 2:2 + W], tot32[:, s, :],
                    _INV_H, Alu.is_equal, Alu.mult)
            nc.vector.tensor_mul(qco[:], qco[:], mf[:])
            nc.vector.tensor_add(qco[:], qco[:], er[:])

            pcoS = None
            if cfg["pcos"]:
                # pcoS[k] = pco[k-1]; shares csB's buffer (csB is dead
                # after the er op; Tile orders the WAR hazard).
                if csB is not None:
                    pcoS = csB
                else:
                    pcoS = pool.tile([P, S, W + 2], F16, tag="pcoS")
                nc.vector.memset(pcoS[:, :, 0:1], 0.0)
                nc.scalar.copy(pcoS[:, :, 1:1 + W], pco[:, :, 0:W])

            # ---- u pipeline, per channel ----
            for c in range(_C):
                usrc = u_ap[b, c, r0:r0 + R, :].rearrange(
                    "(s p) w -> p s w", p=P)
                d = upool.tile([P, S, W + 2], F16, tag="d")
                if cfg["u_conv"] == "fold":
                    # dsub reads f32, writes f16: the cast rides the op.
                    u32p = upool.tile([P, S, W + 2], F32, tag="u32p")
                    if not cfg["pads_once"]:
                        nc.scalar.memzero(u32p[:, :, 0:1])
                        nc.scalar.memzero(u32p[:, :, 1 + W:2 + W])
                    e_uload.dma_start(u32p[:, :, 1:1 + W], usrc)
                    # d[k] = u[k]-u[k-1] = u32p[k+1]-u32p[k], k = 0..W
                    nc.vector.tensor_sub(d[:, :, 0:W + 1],
                                         u32p[:, :, 1:2 + W],
                                         u32p[:, :, 0:1 + W])
                    t1t = upool.tile([P, S, W + 2], F16, tag="t1")
                    t1 = t1t[:]
                else:
                    up = upool.tile([P, S, W + 4], F16, tag="up")
                    nc.scalar.memzero(up[:, :, 0:2])
                    nc.scalar.memzero(up[:, :, 2 + W:4 + W])
                    u32 = upool.tile([P, S, W], F32, tag="u32")
                    e_uload.dma_start(u32[:], usrc)
                    if cfg["u_conv"] == "gp":
                        nc.gpsimd.tensor_copy(up[:, :, 2:2 + W], u32[:])
                    elif cfg["u_conv"] == "dve":
                        nc.vector.tensor_copy(up[:, :, 2:2 + W], u32[:])
                    else:
                        nc.scalar.copy(up[:, :, 2:2 + W], u32[:])
                    # d[k] = u[k]-u[k-1] = up[k+2]-up[k+1], k = 0..W
                    nc.vector.tensor_sub(d[:, :, 0:W + 1],
                                         up[:, :, 2:3 + W],
                                         up[:, :, 1:2 + W])
                    if cfg["t1_alias"]:
                        t1 = up[:, :, 0:W + 2]
                    else:
                        t1t = upool.tile([P, S, W + 2], F16, tag="t1")
                        t1 = t1t[:]

                odst = o_ap[b, c, r0:r0 + R, :].rearrange(
                    "(s p) w -> p s w", p=P)
                o32 = opool.tile([P, S, W], F32, tag="o32")

                if use_pe:
                    # t1[k] = pco[k-1]*d[k]; out[w] = t1[w+1]+qco[w]*d[w]
                    if pcoS is not None:
                        nc.vector.tensor_mul(t1[:, :, 0:W + 1],
                                             pcoS[:, :, 0:W + 1],
                                             d[:, :, 0:W + 1])
                        t1lo = 1
                    else:
                        nc.vector.tensor_mul(t1[:, :, 0:W], pco[:],
                                             d[:, :, 1:1 + W])
                        t1lo = 0
                    nc.vector.tensor_mul(d[:, :, 0:W], qco[:],
                                         d[:, :, 0:W])
                    for s in range(S):
                        pt = ppool.tile([P, W], F32, tag="pt")
                        for j in range(0, W, 512):
                            nc.tensor.matmul(
                                pt[:, j:j + 512], ident[:],
                                t1[:, s, t1lo + j:t1lo + j + 512],
                                start=True, stop=False)
                            nc.tensor.matmul(
                                pt[:, j:j + 512], ident[:],
                                d[:, s, j:j + 512],
                                start=False, stop=True)
                        nc.scalar.copy(o32[:, s, :], pt[:])
                else:
                    # DVE f32-out add (1x) straight into o32
                    if pcoS is not None:
                        nc.vector.tensor_mul(t1[:, :, 0:W + 1],
                                             pcoS[:, :, 0:W + 1],
                                             d[:, :, 0:W + 1])
                        nc.vector.tensor_mul(d[:, :, 0:W], qco[:],
                                             d[:, :, 0:W])
                        nc.vector.tensor_add(o32[:], t1[:, :, 1:1 + W],
                                             d[:, :, 0:W])
                    else:
                        nc.vector.tensor_mul(t1[:, :, 0:W], pco[:],
                                             d[:, :, 1:1 + W])
                        nc.vector.tensor_mul(d[:, :, 0:W], qco[:],
                                             d[:, :, 0:W])
                        nc.vector.tensor_add(o32[:], t1[:, :, 0:W],
                                             d[:, :, 0:W])
                e_odma.dma_start(odst, o32[:])


def _stack():
    from contextlib import ExitStack
    return ExitStack()


_CACHE = {}


def make_fn(cfg=None):
    """Build + jit a sharded callable (u_full, mask_full) -> (out_full,)."""
    cfg = dict(CFG2, **(cfg or {}))
    key = tuple(sorted((k, str(v)) for k, v in cfg.items()))
    if key in _CACHE:
        return _CACHE[key]

    import jax
    from jax.sharding import Mesh, PartitionSpec
    from jax.experimental.shard_map import shard_map
    from concourse import bass2jax, mybir

    nc = _build_nc(cfg)
    bass2jax.install_neuronx_cc_hook()

    partition_name = (nc.partition_id_tensor.name
                      if nc.partition_id_tensor else None)
    in_names = []
    out_names = []
    out_avals = []
    zero_shapes = []
    for alloc in nc.m.functions[0].allocations:
        if not isinstance(alloc, mybir.MemoryLocationSet):
            continue
        name = alloc.memorylocations[0].name
        if alloc.kind == "ExternalInput":
            if name != partition_name:
                in_names.append(name)
        elif alloc.kind == "ExternalOutput":
            out_names.append(name)
            shape = tuple(alloc.tensor_shape)
            dtype = mybir.dt.np(alloc.dtype)
            out_avals.append(jax.core.ShapedArray(shape, dtype))
            zero_shapes.append((shape, dtype))
    n_params = len(in_names)
    all_names = in_names + out_names
    if partition_name is not None:
        all_names = all_names + [partition_name]

    def _jax_body(*args):
        operands = list(args)
        if partition_name is not None:
            operands.append(bass2jax.partition_id_tensor())
        outs = bass2jax._bass_exec_p.bind(
            *operands,
            out_avals=tuple(out_avals),
            in_names=tuple(all_names),
            out_names=tuple(out_names),
            lowering_input_output_aliases=(),
            sim_require_finite=True,
            sim_require_nnan=True,
            nc=nc,
        )
        return tuple(outs)

    devices = jax.devices()[:_NCORES]
    mesh = Mesh(np.asarray(devices), ("core",))
    n_outs = len(out_names)
    sharded = jax.jit(
        shard_map(_jax_body, mesh=mesh,
                  in_specs=(PartitionSpec("core"),) * (n_params + n_outs),
                  out_specs=(PartitionSpec("core"),) * n_outs,
                  check_rep=False),
        donate_argnums=tuple(range(n_params, n_params + n_outs)),
        keep_unused=True,
    )

    name_to_idx = {n: i for i, n in enumerate(in_names)}
    oidx = out_names.index("out")

    def fn(u_full, mask_full, zeros=None):
        args = [None] * n_params
        args[name_to_idx["u"]] = np.ascontiguousarray(
            u_full, dtype=np.float32)
        m_np = np.float16 if cfg["m_dtype"] == "f16" else np.float32
        args[name_to_idx["mask"]] = np.ascontiguousarray(
            mask_full, dtype=m_np).reshape(_B, _H, _W)
        if "ident" in name_to_idx:
            args[name_to_idx["ident"]] = np.tile(
                np.eye(128, dtype=np.float16), (_NCORES, 1))
        zlist = [np.zeros((_NCORES * s[0], *s[1:]), d)
                 for (s, d) in zero_shapes]
        return sharded(*args, *zlist)

    fn.out_index = oidx
    _CACHE[key] = fn
    return fn


def kernel(u, mask):
    fn = make_fn()
    out = np.asarray(fn(u, mask)[fn.out_index])
    return out.reshape(_B, _C, _H, _W)


if __name__ == "__main__":
    rng = np.random.default_rng(0)
    u = rng.standard_normal((_B, _C, _H, _W), dtype=np.float32)
    mask = (rng.random((_B, 1, _H, _W)) < 0.5).astype(np.float32)
    out = kernel(u=u, mask=mask)
    print("out", out.shape, out.dtype, float(np.abs(out).max()))
